# revision 3
# baseline (speedup 1.0000x reference)
"""LogicDense (difflogic) kernel for TRN2, 8 NeuronCores.

Op: out[t, j] = q0[j] + q1[j]*a + q2[j]*b + q3[j]*a*b
    where a = x[t, i0[j]], b = x[t, i1[j]], q = softmax(weight) @ OP_COEFFS.

Sharding: neuron-parallel. Core c owns neurons [c*4096, (c+1)*4096), full batch.
Layout on device: neurons on partitions, batch on the free dim, so the four
affine coefficients are per-partition scalars (ACT scale/bias, DVE tensor ops).
The feature gather is a dma_gather of 8KB rows from a host-transposed xT in
DRAM. Output is written as (4096, 2048) per core; host unshards + transposes.
"""

import numpy as np

import concourse.bacc as bacc
import concourse.tile as tile
from concourse import library_config, mybir
from concourse.bass_utils import run_bass_kernel_spmd

N_CORES = 8
B = 2048
IN_DIM = 8192
OUT_DIM = 32768
NPC = OUT_DIM // N_CORES  # 4096 neurons per core
NBLK = NPC // 128  # 32 blocks of 128 neurons
F32 = mybir.dt.float32
I16 = mybir.dt.int16

OP_COEFFS = np.array([
    [0.0,  0.0,  0.0,  0.0],
    [0.0,  0.0,  0.0,  1.0],
    [0.0,  1.0,  0.0, -1.0],
    [0.0,  1.0,  0.0,  0.0],
    [0.0,  0.0,  1.0, -1.0],
    [0.0,  0.0,  1.0,  0.0],
    [0.0,  1.0,  1.0, -2.0],
    [0.0,  1.0,  1.0, -1.0],
    [1.0, -1.0, -1.0,  1.0],
    [1.0, -1.0, -1.0,  2.0],
    [1.0,  0.0, -1.0,  0.0],
    [1.0,  0.0, -1.0,  1.0],
    [1.0, -1.0,  0.0,  0.0],
    [1.0, -1.0,  0.0,  1.0],
    [1.0,  0.0,  0.0, -1.0],
    [1.0,  0.0,  0.0,  0.0],
], dtype=np.float32)

_NC_CACHE = {}


def _build():
    if "nc" in _NC_CACHE:
        return _NC_CACHE["nc"]
    nc = bacc.Bacc("TRN2", target_bir_lowering=False)
    xt = nc.dram_tensor("xt", [IN_DIM, B], F32, kind="ExternalInput")
    wr = nc.dram_tensor("wr", [128, NBLK, 16], F32, kind="ExternalInput")
    ct = nc.dram_tensor("ct", [128, 4, NBLK, 16], F32, kind="ExternalInput")
    idx0 = nc.dram_tensor("idx0", [128, NBLK, 8], I16, kind="ExternalInput")
    idx1 = nc.dram_tensor("idx1", [128, NBLK, 8], I16, kind="ExternalInput")
    out = nc.dram_tensor("out", [NPC, B], F32, kind="ExternalOutput")

    with tile.TileContext(nc) as tc:
        with (
            tc.tile_pool(name="singles", bufs=1) as singles,
            tc.tile_pool(name="ga", bufs=2) as gapool,
            tc.tile_pool(name="gb", bufs=2) as gbpool,
            tc.tile_pool(name="work", bufs=3) as wpool,
            tc.tile_pool(name="outp", bufs=3) as opool,
        ):
            nc.gpsimd.load_library(library_config.mlp)

            idx0_sb = singles.tile([128, NBLK, 8], I16)
            nc.sync.dma_start(out=idx0_sb, in_=idx0[:, :, :])
            idx1_sb = singles.tile([128, NBLK, 8], I16)
            nc.sync.dma_start(out=idx1_sb, in_=idx1[:, :, :])
            wr_sb = singles.tile([128, NBLK, 16], F32)
            nc.sync.dma_start(out=wr_sb, in_=wr[:, :, :])
            ct_sb = singles.tile([128, 4, NBLK, 16], F32)
            nc.sync.dma_start(out=ct_sb, in_=ct[:, :, :, :])

            # q[p, k, blk] = (sum_c exp(w)[p,blk,c] * C[c,k]) / sum_c exp(w)[p,blk,c]
            e = singles.tile([128, NBLK, 16], F32)
            nc.scalar.activation(
                out=e, in_=wr_sb, func=mybir.ActivationFunctionType.Exp
            )
            s = singles.tile([128, NBLK], F32)
            nc.vector.tensor_reduce(
                out=s, in_=e, axis=mybir.AxisListType.X, op=mybir.AluOpType.add
            )
            r = singles.tile([128, NBLK], F32)
            nc.vector.reciprocal(out=r, in_=s)
            q = singles.tile([128, 4, NBLK], F32)
            for k in range(4):
                tmp = singles.tile([128, NBLK, 16], F32, tag=f"tmp{k}")
                nc.vector.tensor_mul(tmp, e, ct_sb[:, k, :, :])
                nc.vector.tensor_reduce(
                    out=q[:, k, :], in_=tmp,
                    axis=mybir.AxisListType.X, op=mybir.AluOpType.add,
                )
                nc.vector.tensor_mul(q[:, k, :], q[:, k, :], r)

            # Gather granularity: 2 blocks (256 idxs) per dma_gather, a/b on
            # separate SWDGE queues; combine still per 128-neuron block.
            for g in range(NBLK // 2):
                ga = gapool.tile([128, 2, B], F32, tag="ga")
                gb = gbpool.tile([128, 2, B], F32, tag="gb")
                nc.gpsimd.dma_gather(
                    ga, xt[:, :], idx0_sb[:, 2 * g:2 * g + 2, :], 256, 256, B,
                    queue_num=0,
                )
                nc.gpsimd.dma_gather(
                    gb, xt[:, :], idx1_sb[:, 2 * g:2 * g + 2, :], 256, 256, B,
                    queue_num=1,
                )
                for h in range(2):
                    b = 2 * g + h
                    gav = ga[:, h, :]
                    gbv = gb[:, h, :]
                    # v = q3*b + q1 ; w = q2*b + q0 ; out = v*a + w
                    v = wpool.tile([128, B], F32, tag="v")
                    nc.scalar.activation(
                        out=v, in_=gbv, func=mybir.ActivationFunctionType.Identity,
                        bias=q[:, 1, b:b + 1], scale=q[:, 3, b:b + 1],
                    )
                    w = wpool.tile([128, B], F32, tag="w")
                    nc.scalar.activation(
                        out=w, in_=gbv, func=mybir.ActivationFunctionType.Identity,
                        bias=q[:, 0, b:b + 1], scale=q[:, 2, b:b + 1],
                    )
                    u = wpool.tile([128, B], F32, tag="u")
                    nc.vector.tensor_mul(u, gav, v)
                    o = opool.tile([128, B], F32, tag="o")
                    nc.vector.tensor_add(o, u, w)
                    nc.sync.dma_start(out=out[b * 128:(b + 1) * 128, :], in_=o)

    nc.compile()
    _NC_CACHE["nc"] = nc
    return nc


def _wrap_idx(idx_shard):
    # (4096,) -> (128, NBLK, 8) int16, dma_gather wrapped layout:
    # gathered row j of block b reads idx[b*128 + j]; index j lives at
    # partition j%16, slot j//16, replicated across the 8 groups of 16.
    t = idx_shard.reshape(NBLK, 8, 16).astype(np.int16)  # [b, slot, lane]
    return np.ascontiguousarray(np.tile(t.transpose(2, 0, 1), (8, 1, 1)))


def _prep_inputs(x, weight, indices):
    xt = np.ascontiguousarray(np.asarray(x, np.float32).T)  # (8192, 2048)
    weight = np.asarray(weight, np.float32)
    indices = np.asarray(indices)
    ct = np.ascontiguousarray(
        np.broadcast_to(
            OP_COEFFS.T[None, :, None, :], (128, 4, NBLK, 16)
        )
    ).astype(np.float32)
    in_maps = []
    for c in range(N_CORES):
        lo, hi = c * NPC, (c + 1) * NPC
        w_sh = weight[lo:hi]  # (4096, 16)
        wrr = np.ascontiguousarray(w_sh.reshape(NBLK, 128, 16).transpose(1, 0, 2))
        in_maps.append({
            "xt": xt,
            "wr": wrr,
            "ct": ct,
            "idx0": _wrap_idx(indices[0, lo:hi]),
            "idx1": _wrap_idx(indices[1, lo:hi]),
        })
    return in_maps


def _run(x, weight, indices, trace=False):
    nc = _build()
    in_maps = _prep_inputs(x, weight, indices)
    res = run_bass_kernel_spmd(nc, in_maps, core_ids=list(range(N_CORES)),
                               trace=trace)
    shards = [res.results[c]["out"] for c in range(N_CORES)]  # each (4096, 2048)
    full = np.concatenate(shards, axis=0)  # (32768, 2048)
    out = np.ascontiguousarray(full.T).astype(np.float32)  # (2048, 32768)
    return out, res


def kernel(x, weight, indices):
    out, _ = _run(x, weight, indices, trace=False)
    return out
